# revision 2
# baseline (speedup 1.0000x reference)
"""GQA attention (int8-quantized QK^T, RoPE, causal softmax) on 8 TRN2 NeuronCores.

Sharding: tensor-parallel over heads. Core c owns Q heads 4c..4c+3 (Wq cols
512c..512c+512), KV head c (Wk/Wv cols 128c..128c+128), and Wo rows
512c..512c+512. x is replicated (host pre-transposes + casts to bf16). Each
core emits a partial [2048, 4096] bf16 output (its heads' contribution
through Wo); the host sums the 8 partials in float64. No on-device
collectives.

Per-core dataflow (all matmuls bf16; QK^T faithful to the reference's int8
quantization: integer values are produced with the fp32 round-to-even MAGIC
trick exactly as jnp.round does, then the dequant scales absmax/127 (and
SCALE for k) are folded back into the stored bf16 qT/kT, so the scores
matmul emits final logits with only bf16 representation noise ~0.2%):
  A) xT arrives pre-transposed from host via the gpsimd SWDGE queue (so the
     sync HWDGE queue carries ONLY the qi/ki transposes and never backs them
     up behind 2MB loads); weights via the scalar HWDGE queue with wkv first
     (the kv chain is the first real PE work). KV projection emitted before
     Q per s-tile; ScalarE evacuates PSUM; RoPE + absmax-quantize +
     scale-fold on VectorE; DMA-transpose q/k to [hd, s] on sync.
  B) per q-block J and head: scores^T [t, q] = kT-tile.T @ qT-block emit
     final logits; exp on ScalarE straight out of PSUM; causal zeroing of
     diagonal-band tiles on gpsimd post-exp; running probability sum on
     VectorE in bf16; den = gpsimd partition_all_reduce of sump (no PE
     instruction in the normalize chain, so the PE FIFO never stalls on it);
     O^T += V-chunk.T @ P^T. psO uses 4 banks alternating across head-pairs
     so the next pair's PV start never waits on the previous normalize.
  C) out[s, :] += OT-slice.T @ Wo-chunk accumulated over f; one 1MB DMA per
     finished 128-row output block on the scalar queue. C(J) is emitted
     right after B(J) so its matmuls fill TensorE bubbles while B(J+1)
     waits on exp.
"""

import numpy as np

import concourse.bass as bass
import concourse.mybir as mybir
import concourse.tile as tile
from concourse import bacc
from concourse.bass_isa import ReduceOp
from concourse.bass_utils import run_bass_kernel_spmd
from concourse.masks import make_identity

FP = mybir.dt.float32
FR = mybir.dt.float32r
BF = mybir.dt.bfloat16
AL = mybir.AluOpType
AF = mybir.ActivationFunctionType

B, S, D, NH, NKV, HD = 1, 2048, 4096, 32, 8, 128
NCORES = 8
HPC = NH // NCORES          # 4 Q heads per core
FQ = HPC * HD               # 512
SCALE = HD ** -0.5
MAGIC = 3 * 2.0 ** 22       # fp32 round-to-nearest-even magic constant

ST = S // 128               # 16 s-tiles of 128 rows
DC = D // 128               # 32 d-chunks
NJ = S // 512               # 4 q-blocks of 512
XBLK = 256                  # xT columns per DMA block
NXB = S // XBLK             # 8 blocks
NWARM = 56                  # HAM warm-up matmuls (~6us cold, covers wkv DMA)


def build_graph():
    nc = bacc.Bacc(None)
    # all inputs arrive host-swizzled partition-major so every DMA descriptor
    # covers a multi-KB contiguous run (strided layouts fragment into 512B
    # descriptors and make the loads descriptor-bound, ~40us to first matmul)
    xt_e = nc.declare_dram_parameter("xt", [128, NXB * DC * XBLK], BF, isOutput=False)
    wq_e = nc.declare_dram_parameter("wq", [128, DC * FQ], BF, isOutput=False)
    wkv_e = nc.declare_dram_parameter("wkv", [128, DC * 2 * HD], BF, isOutput=False)
    wo_e = nc.declare_dram_parameter("wo", [128, HPC * D], BF, isOutput=False)
    cos_e = nc.declare_dram_parameter("cosr", [128, ST, HD], BF, isOutput=False)
    sin_e = nc.declare_dram_parameter("sinm", [128, ST, HD], BF, isOutput=False)
    out_e = nc.declare_dram_parameter("out", [S, D], BF, isOutput=True)

    xt_r = xt_e[:].rearrange("p (b c s) -> p b c s", b=NXB, c=DC)

    with tile.TileContext(nc, pool_alloc_mode="queue") as tc:
        with (
            tc.tile_pool(name="persist", bufs=1) as pp,
        ):
            # warm the PE HAM clock gate with throwaway matmuls while the
            # first input DMAs land, so the real chains start at full clock
            warm = pp.tile([128, 128], BF)
            nc.gpsimd.memset(warm[:], 0.0)

            qT = pp.tile([128, HPC, S], BF)     # scale-folded Q^T per head
            kT = pp.tile([128, S], BF)          # scale-folded K^T (SCALE folded)
            vn = pp.tile([128, ST, HD], BF)     # V natural, per t-chunk
            OT = pp.tile([128, HPC, S], BF)     # normalized O^T per head
            wo_r = pp.tile([128, HPC, D], BF)

            # ---------------- Phase A: projections, RoPE, quantize+fold
            with (
                tc.tile_pool(name="aw", bufs=1) as awp,
                tc.tile_pool(name="xtp", bufs=2) as xtp,
                tc.tile_pool(name="ab", bufs=4) as ab,
                tc.tile_pool(name="psA", bufs=2, space="PSUM") as psA,
            ):
                wqr = awp.tile([128, DC, FQ], BF)
                wkv = awp.tile([128, DC, 2 * HD], BF)
                cosr = awp.tile([128, ST, HD], BF)
                sinm = awp.tile([128, ST, HD], BF)
                # gpsimd SWDGE queue: x blocks + rope tables + (later) wo.
                # sync HWDGE stays empty for the transposes; scalar HWDGE
                # carries the weights, wkv first since the kv chain is the
                # first matmul consumer.
                xtb_first = xtp.tile([128, DC, XBLK], BF, tag="xtb")
                # first block in d-halves so the first projection chain can
                # begin after 1MB instead of 2MB
                nc.gpsimd.dma_start(xtb_first[:, 0:16, :], xt_r[:, 0, 0:16])
                nc.gpsimd.dma_start(xtb_first[:, 16:32, :], xt_r[:, 0, 16:32])
                nc.gpsimd.dma_start(cosr[:], cos_e[:])
                nc.gpsimd.dma_start(sinm[:], sin_e[:])
                nc.scalar.dma_start(wkv[:], wkv_e[:].rearrange("p (c h) -> p c h", c=DC))
                wq_r = wq_e[:].rearrange("p (c f) -> p c f", c=DC)
                for wc in range(4):
                    nc.scalar.dma_start(wqr[:, wc * 8:(wc + 1) * 8, :],
                                        wq_r[:, wc * 8:(wc + 1) * 8, :])
                # pre-warm the exp table set (one-time ~2.7us) after the
                # critical weight DMAs are already queued on ScalarE
                scratch = pp.tile([128, 1], FP)
                nc.scalar.activation(scratch[:], warm[:, 0:1], AF.Exp)

                # HAM warm-up: throwaway matmuls spanning the ~6us the first
                # input DMAs take to land, so the projection chains start warm
                wps = psA.tile([64, 128], FP, tag="wps")
                for w in range(NWARM):
                    nc.tensor.matmul(wps[:], warm[:, 0:64], warm[:],
                                     start=(w == 0), stop=(w == NWARM - 1))

                for blk in range(NXB):
                    if blk == 0:
                        xtb = xtb_first
                    else:
                        xtb = xtp.tile([128, DC, XBLK], BF, tag="xtb")
                        nc.gpsimd.dma_start(xtb[:], xt_r[:, blk])
                    if blk == 2:
                        # wo prefetch now that the early-critical loads are
                        # done competing for SDMA bandwidth (needed in C only)
                        nc.gpsimd.dma_start(
                            wo_r[:], wo_e[:].rearrange("p (f d) -> p f d", f=HPC))

                    for i in range(XBLK // 128):
                        st_i = blk * (XBLK // 128) + i
                        xts = xtb[:, :, i * 128:(i + 1) * 128]
                        q_ps = psA.tile([128, FQ], FP, tag="qps")
                        kv_ps = psA.tile([128, 2 * HD], FP, tag="kvps")
                        # kv first: wkv is the first weight to land
                        for d in range(DC):
                            nc.tensor.matmul(kv_ps[:], xts[:, d, :], wkv[:, d, :],
                                             start=(d == 0), stop=(d == DC - 1))
                        for d in range(DC):
                            nc.tensor.matmul(q_ps[:], xts[:, d, :], wqr[:, d, :],
                                             start=(d == 0), stop=(d == DC - 1))

                        # ScalarE evacuates PSUM: V natural cast, q/k to fp32
                        nc.scalar.copy(vn[:, st_i, :], kv_ps[:, HD:2 * HD])
                        qf = ab.tile([128, HPC, HD], FP, tag="qf")
                        kf = ab.tile([128, 1, HD], FP, tag="kf")
                        nc.scalar.copy(kf[:], kv_ps[:, 0:HD].unsqueeze(1))
                        nc.scalar.copy(qf[:], q_ps[:].rearrange("p (h d) -> p h d", h=HPC))

                        # RoPE + quantize + scale-fold: q (4 heads) and k (1)
                        qi = ab.tile([128, HPC, HD], BF, tag="qi")
                        ki = ab.tile([128, 1, HD], BF, tag="ki")
                        co = cosr[:, st_i, :]
                        si = sinm[:, st_i, :]
                        for (src, nh, i8out, kscale) in (
                                (kf, 1, ki, SCALE), (qf, HPC, qi, 1.0)):
                            rr = ab.tile([128, nh, HD], FP, tag=f"rr{nh}")
                            t2 = ab.tile([128, nh, HD], FP, tag=f"t2{nh}")
                            am = ab.tile([128, nh], FP, tag=f"am{nh}")
                            am2 = ab.tile([128, nh], FP, tag=f"am2{nh}")
                            sc = ab.tile([128, nh], FP, tag=f"sc{nh}")
                            u = ab.tile([128, nh], FP, tag=f"u{nh}")
                            cob = co.unsqueeze(1).broadcast_to([128, nh, HD])
                            sib = si.unsqueeze(1).broadcast_to([128, nh, HD])
                            nc.vector.tensor_mul(rr[:], src[:], cob)
                            nc.vector.tensor_mul(t2[:, :, 0:64], src[:, :, 64:HD], sib[:, :, 0:64])
                            nc.vector.tensor_mul(t2[:, :, 64:HD], src[:, :, 0:64], sib[:, :, 64:HD])
                            nc.vector.tensor_add(rr[:], rr[:], t2[:])
                            nc.vector.tensor_reduce(am[:], rr[:], axis=mybir.AxisListType.X,
                                                    op=AL.max, apply_absolute_value=True)
                            nc.vector.tensor_scalar_max(am[:], am[:], 1e-5)
                            # am2 = am/127 ; sc = 127/am (Newton-refined)
                            nc.vector.tensor_scalar_mul(am2[:], am[:], 1.0 / 127.0)
                            nc.vector.reciprocal_approx_fast(sc[:], am2[:])
                            nc.vector.tensor_mul(u[:], am2[:], sc[:])
                            nc.vector.tensor_scalar(u[:], u[:], -1.0, 2.0, op0=AL.mult, op1=AL.add)
                            nc.vector.tensor_mul(sc[:], sc[:], u[:])
                            if kscale != 1.0:
                                amk = ab.tile([128, nh], FP, tag=f"amk{nh}")
                                nc.vector.tensor_scalar_mul(amk[:], am2[:], kscale)
                                unscale = amk
                            else:
                                unscale = am2
                            for h in range(nh):
                                nc.vector.tensor_scalar(rr[:, h, :], rr[:, h, :],
                                                        sc[:, h:h + 1], MAGIC,
                                                        op0=AL.mult, op1=AL.add)
                                nc.vector.tensor_scalar(i8out[:, h, :], rr[:, h, :],
                                                        MAGIC, unscale[:, h:h + 1],
                                                        op0=AL.subtract, op1=AL.mult)

                        # DMA-transpose folded q/k into [hd, s] layout on the
                        # sync queue (SBUF -> SBUF via the X-bar; the sync
                        # queue carries nothing else so these never back up
                        # behind bulk loads)
                        ssl = slice(st_i * 128, (st_i + 1) * 128)
                        nc.sync.dma_start(kT[:, ssl], ki[:, 0, :],
                                          transpose=True)
                        for h in range(HPC):
                            nc.sync.dma_start(qT[:, h, ssl], qi[:, h, :],
                                              transpose=True)

            # ---------------- Phases B+C interleaved per q-block J
            with (
                tc.tile_pool(name="bt", bufs=6) as bt,
                tc.tile_pool(name="bd", bufs=2) as bd,
                tc.tile_pool(name="ct", bufs=2) as ct,
                tc.tile_pool(name="psSC", bufs=2, space="PSUM") as psSC,
                tc.tile_pool(name="psO", bufs=1, space="PSUM") as psO,
                tc.tile_pool(name="psC", bufs=2, space="PSUM") as psC,
            ):
                ct_state = {}

                def emit_c_group(J, k, ct_pool=None, psc_pool=None):
                    # one output-projection column group: 4 accumulating
                    # matmuls + one PSUM evacuation; one 1MB DMA per st row
                    ct_pool = ct_pool if ct_pool is not None else ct
                    psc_pool = psc_pool if psc_pool is not None else psC
                    st_i = 4 * J + k // 8
                    dbl = k % 8
                    ssl = slice(st_i * 128, (st_i + 1) * 128)
                    if dbl == 0:
                        ot_new = ct_pool.tile([128, D], BF, tag="ot")
                        ct_state["ot"] = ot_new
                    ot_sb = ct_state["ot"]
                    wo_ps = psc_pool.tile([128, 512], FP, tag="wo")
                    for f in range(HPC):
                        nc.tensor.matmul(wo_ps[:], OT[:, f, ssl],
                                         wo_r[:, f, dbl * 512:(dbl + 1) * 512],
                                         start=(f == 0), stop=(f == HPC - 1))
                    if dbl % 3 == 2:
                        nc.scalar.copy(ot_sb[:, dbl * 512:(dbl + 1) * 512], wo_ps[:])
                    else:
                        nc.vector.tensor_copy(ot_sb[:, dbl * 512:(dbl + 1) * 512], wo_ps[:])
                    if dbl == 7:
                        nc.scalar.dma_start(out_e[ssl, :], ot_sb[:])

                for J in range(NJ):
                    nlive = 4 * J + 4
                    Jsl = slice(J * 512, (J + 1) * 512)
                    # interleave the previous q-block's output projection into
                    # this block's attention loop: the PE engine FIFO is strict,
                    # so C matmuls must be emitted inside B's exp-paced stretches
                    # to fill them
                    c_queue = list(range(32)) if J > 0 else []
                    n_iters = 2 * nlive
                    it = 0
                    emitted = 0
                    # diagonal-band tiles first: their gpsimd causal selects
                    # pipeline against the clean tiles that follow instead of
                    # stalling the accumulation tail
                    ti_order = list(range(4 * J, nlive)) + list(range(0, 4 * J))
                    for hp in range(HPC // 2):
                        # two single-head streams zipped at tile granularity:
                        # one head's matmuls hide the other's exp round-trip
                        # latency without needing paired PSUM tiles
                        h0, h1 = 2 * hp, 2 * hp + 1
                        # psO banks alternate across head-pairs so this pair's
                        # first PV (start=True) never waits on the previous
                        # pair's normalize read
                        tg = 2 * (hp % 2)
                        oT0 = psO.tile([128, 512], FP, tag=f"o{tg}")
                        oT1 = psO.tile([128, 512], FP, tag=f"o{tg + 1}")
                        # bf16 probability accumulators: 2x DVE rate; the
                        # ~0.3% bf16 accumulation noise on den is well within
                        # the error budget
                        sump0 = bd.tile([128, 512], BF, tag="sump0")
                        sump1 = bd.tile([128, 512], BF, tag="sump1")
                        for idx, ti in enumerate(ti_order):
                            # columns below off are fully above the causal
                            # diagonal for this tile; skip them everywhere
                            off = max(0, ti * 128 - J * 512)
                            kts = kT[:, ti * 128:(ti + 1) * 128]
                            qsl = slice(J * 512 + off, (J + 1) * 512)
                            pts = []
                            for g, hh in ((0, h0), (1, h1)):
                                sc_ps = psSC.tile([128, 512], FP, tag="sc")
                                nc.tensor.matmul(sc_ps[:, off:], kts, qT[:, hh, qsl])
                                pt = bt.tile([128, 512], BF, tag="pt")
                                nc.scalar.activation(pt[:, off:], sc_ps[:, off:], AF.Exp)
                                if ti >= 4 * J:
                                    nc.gpsimd.affine_select(
                                        out=pt[:, off:], in_=pt[:, off:],
                                        compare_op=AL.is_ge, fill=0.0,
                                        base=0, channel_multiplier=-1,
                                        pattern=[[1, 512 - off]])
                                pts.append(pt)
                            for (pt, oT, sump) in ((pts[0], oT0, sump0),
                                                   (pts[1], oT1, sump1)):
                                nc.tensor.matmul(oT[:, off:], vn[:, ti, :], pt[:, off:],
                                                 start=(idx == 0), stop=(idx == nlive - 1))
                                if idx == 0:
                                    nc.vector.tensor_copy(sump[:], pt[:])
                                else:
                                    nc.vector.tensor_add(sump[:, off:], sump[:, off:],
                                                         pt[:, off:])
                            it += 1
                            while c_queue and emitted < it * 32 // n_iters:
                                emit_c_group(J - 1, c_queue.pop(0))
                                emitted += 1
                        for (oT, sump, hh) in ((oT0, sump0, h0), (oT1, sump1, h1)):
                            # den via gpsimd all-reduce: no PE instruction in
                            # the normalize chain, so the PE FIFO never stalls
                            ar = bd.tile([128, 512], FP, tag="ar")
                            nc.gpsimd.partition_all_reduce(ar[:], sump[:], 128,
                                                           ReduceOp.add)
                            rcp = bd.tile([128, 512], FP, tag="rcp")
                            nc.vector.reciprocal_approx_fast(rcp[:], ar[:])
                            nc.vector.tensor_mul(OT[:, hh, Jsl], oT[:], rcp[:])

                    # drain any of last block's C groups the cadence missed
                    while c_queue:
                        emit_c_group(J - 1, c_queue.pop(0))

            # final q-block's output projection: fresh PSUM scope (the B pools
            # are dead now) so it can deep-buffer and run at full PE rate
            with (
                tc.tile_pool(name="ct2", bufs=2) as ct2,
                tc.tile_pool(name="psC2", bufs=4, space="PSUM") as psC2,
            ):
                for k in range(32):
                    emit_c_group(NJ - 1, k, ct_pool=ct2, psc_pool=psC2)

    nc.compile()
    return nc


def _pmajor(w):
    # [K*128, N] -> [128, K*N] with each partition's K rows contiguous
    k128, n = w.shape
    return np.ascontiguousarray(
        w.reshape(k128 // 128, 128, n).transpose(1, 0, 2).reshape(128, -1))


def make_in_maps(x, Wq, Wk, Wv, Wo, cos, sin):
    import ml_dtypes
    bf = ml_dtypes.bfloat16
    x2 = np.asarray(x, np.float32).reshape(S, D).astype(bf)
    # [p, blk, c, s_local] so each xtb block is 16KB contiguous per partition
    xt = np.ascontiguousarray(
        x2.reshape(NXB, XBLK, DC, 128).transpose(3, 0, 2, 1).reshape(128, -1))
    cosr = np.ascontiguousarray(
        np.asarray(cos, np.float32).reshape(ST, 128, HD).transpose(1, 0, 2).astype(bf))
    sinm_f = np.asarray(sin, np.float32).copy()
    sinm_f[:, :64] *= -1.0
    sinm = np.ascontiguousarray(
        sinm_f.reshape(ST, 128, HD).transpose(1, 0, 2).astype(bf))
    Wq = np.asarray(Wq, np.float32)
    Wk = np.asarray(Wk, np.float32)
    Wv = np.asarray(Wv, np.float32)
    Wo = np.asarray(Wo, np.float32)
    in_maps = []
    for c in range(NCORES):
        wkv = np.concatenate(
            [Wk[:, c * HD:(c + 1) * HD], Wv[:, c * HD:(c + 1) * HD]], axis=1)
        in_maps.append({
            "xt": xt,
            "wq": _pmajor(Wq[:, c * FQ:(c + 1) * FQ].astype(bf)),
            "wkv": _pmajor(wkv.astype(bf)),
            "wo": _pmajor(Wo[c * FQ:(c + 1) * FQ, :].astype(bf)),
            "cosr": cosr,
            "sinm": sinm,
        })
    return in_maps


_CACHE = {}


def kernel(x, Wq, Wk, Wv, Wo, cos, sin):
    in_maps = make_in_maps(x, Wq, Wk, Wv, Wo, cos, sin)
    if "nc" not in _CACHE:
        _CACHE["nc"] = build_graph()
    try:
        res = run_bass_kernel_spmd(_CACHE["nc"], in_maps, core_ids=list(range(NCORES)))
    except Exception:
        # transient NRT/device hiccups (e.g. EXEC_UNIT_UNRECOVERABLE) usually
        # clear on a fresh attempt
        import time
        time.sleep(20)
        res = run_bass_kernel_spmd(_CACHE["nc"], in_maps, core_ids=list(range(NCORES)))
    out = np.zeros((S, D), np.float64)
    for r in res.results:
        out += np.asarray(r["out"], np.float64)
    return out.astype(np.float32).reshape(B, S, D)


# revision 4
# speedup vs baseline: 1.0410x; 1.0410x over previous
"""GQA attention (int8-quantized QK^T, RoPE, causal softmax) on 8 TRN2 NeuronCores.

Sharding: tensor-parallel over heads. Core c owns Q heads 4c..4c+3 (Wq cols
512c..512c+512), KV head c (Wk/Wv cols 128c..128c+128), and Wo rows
512c..512c+512. x is replicated (host pre-transposes + casts to bf16). Each
core emits a partial [2048, 4096] bf16 output (its heads' contribution
through Wo); the host sums the 8 partials in float64. No on-device
collectives.

Per-core dataflow (all matmuls bf16; QK^T faithful to the reference's int8
quantization: integer values are produced with the fp32 round-to-even MAGIC
trick exactly as jnp.round does, then the dequant scales absmax/127 (and
SCALE for k) are folded back into the stored bf16 qT/kT, so the scores
matmul emits final logits with only bf16 representation noise ~0.2%):
  A) x blocks arrive on the scalar HWDGE queue behind the weights (wkv
     first: the kv chain is the first PE consumer); the first block on the
     idle sync queue in d-halves; sync otherwise carries ONLY the qi/ki
     transposes so they never back up behind bulk loads; cos/sin on the
     gpsimd SWDGE queue. KV projection before Q per s-tile; ScalarE
     evacuates PSUM; RoPE + absmax-quantize + scale-fold on VectorE;
     DMA-transpose q/k to [hd, s] on sync.
  B) per q-block J and zipped head-pair: scores^T [t, q] for BOTH heads land
     in one 2-bank PSUM slab, then a single wide exp on ScalarE covers the
     pair (halves the exp instruction count; the garbage gap between halves
     of diagonal-band tiles is exp'd harmlessly and never read); causal
     zeroing of band tiles on gpsimd post-exp; running probability sum on
     VectorE in bf16; den = ones.T @ sump matmul, deferred behind stashed
     output-projection groups so the PE FIFO never waits on the sump chain;
     O^T += V-chunk.T @ P^T.
  C) out[s, :] += OT-slice.T @ Wo-chunk accumulated over f, DMA out (bf16)
     per half-row on the Scalar queue. C(J) is emitted right after B(J) so
     its matmuls fill TensorE bubbles while B(J+1) waits on exp; a few
     groups are held back to cover each pair's den/normalize chain.
"""

import numpy as np

import concourse.bass as bass
import concourse.mybir as mybir
import concourse.tile as tile
from concourse import bacc
from concourse.bass_utils import run_bass_kernel_spmd
from concourse.masks import make_identity

FP = mybir.dt.float32
FR = mybir.dt.float32r
BF = mybir.dt.bfloat16
AL = mybir.AluOpType
AF = mybir.ActivationFunctionType

B, S, D, NH, NKV, HD = 1, 2048, 4096, 32, 8, 128
NCORES = 8
HPC = NH // NCORES          # 4 Q heads per core
FQ = HPC * HD               # 512
SCALE = HD ** -0.5
MAGIC = 3 * 2.0 ** 22       # fp32 round-to-nearest-even magic constant

ST = S // 128               # 16 s-tiles of 128 rows
DC = D // 128               # 32 d-chunks
NJ = S // 512               # 4 q-blocks of 512
XBLK = 256                  # xT columns per DMA block
NXB = S // XBLK             # 8 blocks
NWARM = 96                  # HAM warm-up matmuls (~10us cold, covers wkv+xt0)


def build_graph():
    nc = bacc.Bacc(None)
    # all inputs arrive host-swizzled partition-major so every DMA descriptor
    # covers a multi-KB contiguous run (strided layouts fragment into 512B
    # descriptors and make the loads descriptor-bound, ~40us to first matmul)
    xt_e = nc.declare_dram_parameter("xt", [128, NXB * DC * XBLK], BF, isOutput=False)
    wq_e = nc.declare_dram_parameter("wq", [128, DC * FQ], BF, isOutput=False)
    wkv_e = nc.declare_dram_parameter("wkv", [128, DC * 2 * HD], BF, isOutput=False)
    wo_e = nc.declare_dram_parameter("wo", [128, HPC * D], BF, isOutput=False)
    cos_e = nc.declare_dram_parameter("cosr", [128, ST, HD], BF, isOutput=False)
    sin_e = nc.declare_dram_parameter("sinm", [128, ST, HD], BF, isOutput=False)
    out_e = nc.declare_dram_parameter("out", [S, D], BF, isOutput=True)

    xt_r = xt_e[:].rearrange("p (b c s) -> p b c s", b=NXB, c=DC)

    with tile.TileContext(nc, pool_alloc_mode="queue") as tc:
        with (
            tc.tile_pool(name="persist", bufs=1) as pp,
        ):
            ones1 = pp.tile([128, 1], BF)       # den stationary (M=1)
            nc.gpsimd.memset(ones1[:], 1.0)
            # warm the PE HAM clock gate with throwaway matmuls while the
            # first input DMAs land, so the real chains start at full clock
            warm = pp.tile([128, 128], BF)
            nc.gpsimd.memset(warm[:], 0.0)

            qT = pp.tile([128, HPC, S], BF)     # scale-folded Q^T per head
            kT = pp.tile([128, S], BF)          # scale-folded K^T (SCALE folded)
            vn = pp.tile([128, ST, HD], BF)     # V natural, per t-chunk
            OT = pp.tile([128, HPC, S], BF)     # normalized O^T per head
            wo_r = pp.tile([128, HPC, D], BF)

            # ---------------- Phase A: projections, RoPE, quantize+fold
            with (
                tc.tile_pool(name="aw", bufs=1) as awp,
                tc.tile_pool(name="xtp", bufs=2) as xtp,
                tc.tile_pool(name="ab", bufs=4) as ab,
                tc.tile_pool(name="psA", bufs=2, space="PSUM") as psA,
            ):
                wqr = awp.tile([128, DC, FQ], BF)
                wkv = awp.tile([128, DC, 2 * HD], BF)
                cosr = awp.tile([128, ST, HD], BF)
                sinm = awp.tile([128, ST, HD], BF)
                # startup-critical loads: wkv on scalar HWDGE, first x block
                # on the idle sync HWDGE, rope tables on gpsimd SWDGE — three
                # rings pulling concurrently so the kv chain can start ~12us
                xtb_first = xtp.tile([128, DC, XBLK], BF, tag="xtb")
                # first block in d-halves so the first projection chain can
                # begin after 1MB instead of 2MB
                nc.sync.dma_start(xtb_first[:, 0:16, :], xt_r[:, 0, 0:16])
                nc.sync.dma_start(xtb_first[:, 16:32, :], xt_r[:, 0, 16:32])
                nc.scalar.dma_start(wkv[:], wkv_e[:].rearrange("p (c h) -> p c h", c=DC))
                nc.gpsimd.dma_start(cosr[:], cos_e[:])
                nc.gpsimd.dma_start(sinm[:], sin_e[:])
                wq_r = wq_e[:].rearrange("p (c f) -> p c f", c=DC)
                for wc in range(4):
                    nc.scalar.dma_start(wqr[:, wc * 8:(wc + 1) * 8, :],
                                        wq_r[:, wc * 8:(wc + 1) * 8, :])
                # pre-warm the exp table set (one-time ~2.7us) after the
                # critical weight DMAs are already queued on ScalarE
                scratch = pp.tile([128, 1], FP)
                nc.scalar.activation(scratch[:], warm[:, 0:1], AF.Exp)

                # HAM warm-up: throwaway matmuls spanning the ~10us the first
                # input DMAs take to land, so the projection chains start warm
                wps = psA.tile([64, 128], FP, tag="wps")
                for w in range(NWARM):
                    nc.tensor.matmul(wps[:], warm[:, 0:64], warm[:],
                                     start=(w == 0), stop=(w == NWARM - 1))

                for blk in range(NXB):
                    if blk == 0:
                        xtb = xtb_first
                    else:
                        # later x blocks ride the scalar HWDGE ring behind the
                        # weights; by steady state they arrive 2 tiles early
                        xtb = xtp.tile([128, DC, XBLK], BF, tag="xtb")
                        nc.scalar.dma_start(xtb[:], xt_r[:, blk])
                    if blk == 2:
                        # wo prefetch now that the early-critical loads are
                        # done competing for SDMA bandwidth (needed in C only)
                        nc.gpsimd.dma_start(
                            wo_r[:], wo_e[:].rearrange("p (f d) -> p f d", f=HPC))

                    for i in range(XBLK // 128):
                        st_i = blk * (XBLK // 128) + i
                        xts = xtb[:, :, i * 128:(i + 1) * 128]
                        q_ps = psA.tile([128, FQ], FP, tag="qps")
                        kv_ps = psA.tile([128, 2 * HD], FP, tag="kvps")
                        # kv first: wkv is the first weight to land
                        for d in range(DC):
                            nc.tensor.matmul(kv_ps[:], xts[:, d, :], wkv[:, d, :],
                                             start=(d == 0), stop=(d == DC - 1))
                        for d in range(DC):
                            nc.tensor.matmul(q_ps[:], xts[:, d, :], wqr[:, d, :],
                                             start=(d == 0), stop=(d == DC - 1))

                        # ScalarE evacuates PSUM: V natural cast, q/k to fp32
                        nc.scalar.copy(vn[:, st_i, :], kv_ps[:, HD:2 * HD])
                        qf = ab.tile([128, HPC, HD], FP, tag="qf")
                        kf = ab.tile([128, 1, HD], FP, tag="kf")
                        nc.scalar.copy(kf[:], kv_ps[:, 0:HD].unsqueeze(1))
                        nc.scalar.copy(qf[:], q_ps[:].rearrange("p (h d) -> p h d", h=HPC))

                        # RoPE + quantize + scale-fold: q (4 heads) and k (1)
                        qi = ab.tile([128, HPC, HD], BF, tag="qi")
                        ki = ab.tile([128, 1, HD], BF, tag="ki")
                        co = cosr[:, st_i, :]
                        si = sinm[:, st_i, :]
                        for (src, nh, i8out, kscale) in (
                                (kf, 1, ki, SCALE), (qf, HPC, qi, 1.0)):
                            rr = ab.tile([128, nh, HD], FP, tag=f"rr{nh}")
                            t2 = ab.tile([128, nh, HD], FP, tag=f"t2{nh}")
                            am = ab.tile([128, nh], FP, tag=f"am{nh}")
                            am2 = ab.tile([128, nh], FP, tag=f"am2{nh}")
                            sc = ab.tile([128, nh], FP, tag=f"sc{nh}")
                            u = ab.tile([128, nh], FP, tag=f"u{nh}")
                            cob = co.unsqueeze(1).broadcast_to([128, nh, HD])
                            sib = si.unsqueeze(1).broadcast_to([128, nh, HD])
                            nc.vector.tensor_mul(rr[:], src[:], cob)
                            nc.vector.tensor_mul(t2[:, :, 0:64], src[:, :, 64:HD], sib[:, :, 0:64])
                            nc.vector.tensor_mul(t2[:, :, 64:HD], src[:, :, 0:64], sib[:, :, 64:HD])
                            nc.vector.tensor_add(rr[:], rr[:], t2[:])
                            nc.vector.tensor_reduce(am[:], rr[:], axis=mybir.AxisListType.X,
                                                    op=AL.max, apply_absolute_value=True)
                            nc.vector.tensor_scalar_max(am[:], am[:], 1e-5)
                            # am2 = am/127 ; sc = 127/am (Newton-refined)
                            nc.vector.tensor_scalar_mul(am2[:], am[:], 1.0 / 127.0)
                            nc.vector.reciprocal_approx_fast(sc[:], am2[:])
                            nc.vector.tensor_mul(u[:], am2[:], sc[:])
                            nc.vector.tensor_scalar(u[:], u[:], -1.0, 2.0, op0=AL.mult, op1=AL.add)
                            nc.vector.tensor_mul(sc[:], sc[:], u[:])
                            if kscale != 1.0:
                                amk = ab.tile([128, nh], FP, tag=f"amk{nh}")
                                nc.vector.tensor_scalar_mul(amk[:], am2[:], kscale)
                                unscale = amk
                            else:
                                unscale = am2
                            for h in range(nh):
                                nc.vector.tensor_scalar(rr[:, h, :], rr[:, h, :],
                                                        sc[:, h:h + 1], MAGIC,
                                                        op0=AL.mult, op1=AL.add)
                                nc.vector.tensor_scalar(i8out[:, h, :], rr[:, h, :],
                                                        MAGIC, unscale[:, h:h + 1],
                                                        op0=AL.subtract, op1=AL.mult)

                        # DMA-transpose folded q/k into [hd, s] layout on the
                        # sync queue (SBUF -> SBUF via the X-bar; the sync
                        # queue carries nothing else after the first x block,
                        # so these never back up behind bulk loads)
                        ssl = slice(st_i * 128, (st_i + 1) * 128)
                        nc.sync.dma_start(kT[:, ssl], ki[:, 0, :],
                                          transpose=True)
                        for h in range(HPC):
                            nc.sync.dma_start(qT[:, h, ssl], qi[:, h, :],
                                              transpose=True)

            # ---------------- Phases B+C interleaved per q-block J
            with (
                tc.tile_pool(name="bt", bufs=4) as bt,
                tc.tile_pool(name="bd", bufs=2) as bd,
                tc.tile_pool(name="ct", bufs=3) as ct,
                tc.tile_pool(name="psSC", bufs=1, space="PSUM") as psSC,
                tc.tile_pool(name="psO", bufs=1, space="PSUM") as psO,
                tc.tile_pool(name="psDen", bufs=1, space="PSUM") as psDen,
                tc.tile_pool(name="psC", bufs=3, space="PSUM") as psC,
            ):
                ct_state = {}

                def emit_c_group(J, k, ct_pool=None, psc_pool=None):
                    # one output-projection column group: 4 accumulating
                    # matmuls + one PSUM evacuation; DMA per (st, half)
                    ct_pool = ct_pool if ct_pool is not None else ct
                    psc_pool = psc_pool if psc_pool is not None else psC
                    st_i = 4 * J + k // 8
                    half = (k % 8) // 4
                    dbl = k % 4
                    db = half * 4 + dbl
                    ssl = slice(st_i * 128, (st_i + 1) * 128)
                    if dbl == 0:
                        ot_new = ct_pool.tile([128, D // 2], BF, tag="ot")
                        ct_state["ot"] = ot_new
                    ot_sb = ct_state["ot"]
                    wo_ps = psc_pool.tile([128, 512], FP, tag="wo")
                    for f in range(HPC):
                        nc.tensor.matmul(wo_ps[:], OT[:, f, ssl],
                                         wo_r[:, f, db * 512:(db + 1) * 512],
                                         start=(f == 0), stop=(f == HPC - 1))
                    if db % 2 == 0:
                        nc.scalar.copy(ot_sb[:, dbl * 512:(dbl + 1) * 512], wo_ps[:])
                    else:
                        nc.vector.tensor_copy(ot_sb[:, dbl * 512:(dbl + 1) * 512], wo_ps[:])
                    if dbl == 3:
                        nc.scalar.dma_start(
                            out_e[ssl, half * (D // 2):(half + 1) * (D // 2)],
                            ot_sb[:])

                for J in range(NJ):
                    nlive = 4 * J + 4
                    Jsl = slice(J * 512, (J + 1) * 512)
                    # interleave the previous q-block's output projection into
                    # this block's attention loop: the PE engine FIFO is strict,
                    # so C matmuls must be emitted inside B's exp-paced stretches
                    # to fill them; ~8 groups are held back for the pair-end
                    # den/normalize chains
                    c_queue = list(range(32)) if J > 0 else []
                    n_iters = 2 * nlive
                    it = 0
                    emitted = 0

                    def emit_some(n):
                        nonlocal emitted
                        for _ in range(n):
                            if not c_queue:
                                return
                            emit_c_group(J - 1, c_queue.pop(0))
                            emitted += 1

                    # diagonal-band tiles first: their gpsimd causal selects
                    # pipeline against the clean tiles that follow instead of
                    # stalling the accumulation tail
                    ti_order = list(range(4 * J, nlive)) + list(range(0, 4 * J))
                    for hp in range(HPC // 2):
                        # two single-head streams zipped at tile granularity:
                        # one head's matmuls hide the other's exp round-trip
                        # latency; both heads' scores share one 2-bank PSUM
                        # slab so a single wide exp serves the pair
                        h0, h1 = 2 * hp, 2 * hp + 1
                        oT0 = psO.tile([128, 512], FP, tag="o0")
                        oT1 = psO.tile([128, 512], FP, tag="o1")
                        # bf16 probability accumulators: 2x DVE rate; the
                        # ~0.3% bf16 accumulation noise on den is well within
                        # the error budget
                        sump0 = bd.tile([128, 512], BF, tag="sump0")
                        sump1 = bd.tile([128, 512], BF, tag="sump1")
                        for idx, ti in enumerate(ti_order):
                            # columns below off are fully above the causal
                            # diagonal for this tile; skip them everywhere
                            off = max(0, ti * 128 - J * 512)
                            kts = kT[:, ti * 128:(ti + 1) * 128]
                            qsl = slice(J * 512 + off, (J + 1) * 512)
                            sc_ps = psSC.tile([128, 1024], FP, tag="sc")
                            nc.tensor.matmul(sc_ps[:, off:512], kts, qT[:, h0, qsl])
                            nc.tensor.matmul(sc_ps[:, 512 + off:1024], kts,
                                             qT[:, h1, qsl])
                            pt = bt.tile([128, 1024], BF, tag="pt")
                            # one exp covers both heads; for band tiles the
                            # never-written gap [512:512+off] is exp'd too —
                            # harmless garbage that nothing reads
                            nc.scalar.activation(pt[:, off:], sc_ps[:, off:], AF.Exp)
                            if ti >= 4 * J:
                                for g in (0, 1):
                                    nc.gpsimd.affine_select(
                                        out=pt[:, g * 512 + off:(g + 1) * 512],
                                        in_=pt[:, g * 512 + off:(g + 1) * 512],
                                        compare_op=AL.is_ge, fill=0.0,
                                        base=0, channel_multiplier=-1,
                                        pattern=[[1, 512 - off]])
                            for g, (oT, sump) in ((0, (oT0, sump0)),
                                                  (1, (oT1, sump1))):
                                ph = pt[:, g * 512:(g + 1) * 512]
                                nc.tensor.matmul(oT[:, off:], vn[:, ti, :], ph[:, off:],
                                                 start=(idx == 0), stop=(idx == nlive - 1))
                                if idx == 0:
                                    nc.vector.tensor_copy(sump[:], ph)
                                else:
                                    nc.vector.tensor_add(sump[:, off:], sump[:, off:],
                                                         ph[:, off:])
                            it += 1
                            while c_queue and emitted < it * 24 // n_iters:
                                emit_c_group(J - 1, c_queue.pop(0))
                                emitted += 1
                        # pair-end den/normalize: cover the sump->den matmul
                        # dependency and the den-bank recycle with held-back C
                        # groups so the PE FIFO keeps streaming
                        emit_some(2)
                        for (oT, sump, hh) in ((oT0, sump0, h0), (oT1, sump1, h1)):
                            den_ps = psDen.tile([1, 512], FP, tag="den")
                            nc.tensor.matmul(den_ps[:], ones1[:], sump[:])
                            emit_some(1)
                            denr = bd.tile([1, 512], FP, tag="denr")
                            nc.vector.reciprocal_approx_fast(denr[:], den_ps[:])
                            dnb = bd.tile([128, 512], FP, tag="dnb")
                            nc.gpsimd.partition_broadcast(dnb[:], denr[:])
                            nc.vector.tensor_mul(OT[:, hh, Jsl], oT[:], dnb[:])
                        emit_some(1)

                    # drain any of last block's C groups the cadence missed
                    while c_queue:
                        emit_c_group(J - 1, c_queue.pop(0))

            # final q-block's output projection: fresh PSUM scope (the B pools
            # are dead now) so it can deep-buffer and run at full PE rate
            with (
                tc.tile_pool(name="ct2", bufs=3) as ct2,
                tc.tile_pool(name="psC2", bufs=4, space="PSUM") as psC2,
            ):
                for k in range(32):
                    emit_c_group(NJ - 1, k, ct_pool=ct2, psc_pool=psC2)

    nc.compile()
    return nc


def _pmajor(w):
    # [K*128, N] -> [128, K*N] with each partition's K rows contiguous
    k128, n = w.shape
    return np.ascontiguousarray(
        w.reshape(k128 // 128, 128, n).transpose(1, 0, 2).reshape(128, -1))


def make_in_maps(x, Wq, Wk, Wv, Wo, cos, sin):
    import ml_dtypes
    bf = ml_dtypes.bfloat16
    x2 = np.asarray(x, np.float32).reshape(S, D).astype(bf)
    # [p, blk, c, s_local] so each xtb block is 16KB contiguous per partition
    xt = np.ascontiguousarray(
        x2.reshape(NXB, XBLK, DC, 128).transpose(3, 0, 2, 1).reshape(128, -1))
    cosr = np.ascontiguousarray(
        np.asarray(cos, np.float32).reshape(ST, 128, HD).transpose(1, 0, 2).astype(bf))
    sinm_f = np.asarray(sin, np.float32).copy()
    sinm_f[:, :64] *= -1.0
    sinm = np.ascontiguousarray(
        sinm_f.reshape(ST, 128, HD).transpose(1, 0, 2).astype(bf))
    Wq = np.asarray(Wq, np.float32)
    Wk = np.asarray(Wk, np.float32)
    Wv = np.asarray(Wv, np.float32)
    Wo = np.asarray(Wo, np.float32)
    in_maps = []
    for c in range(NCORES):
        wkv = np.concatenate(
            [Wk[:, c * HD:(c + 1) * HD], Wv[:, c * HD:(c + 1) * HD]], axis=1)
        in_maps.append({
            "xt": xt,
            "wq": _pmajor(Wq[:, c * FQ:(c + 1) * FQ].astype(bf)),
            "wkv": _pmajor(wkv.astype(bf)),
            "wo": _pmajor(Wo[c * FQ:(c + 1) * FQ, :].astype(bf)),
            "cosr": cosr,
            "sinm": sinm,
        })
    return in_maps


_CACHE = {}


def kernel(x, Wq, Wk, Wv, Wo, cos, sin):
    in_maps = make_in_maps(x, Wq, Wk, Wv, Wo, cos, sin)
    if "nc" not in _CACHE:
        _CACHE["nc"] = build_graph()
    try:
        res = run_bass_kernel_spmd(_CACHE["nc"], in_maps, core_ids=list(range(NCORES)))
    except Exception:
        # transient NRT/device hiccups (e.g. EXEC_UNIT_UNRECOVERABLE) usually
        # clear on a fresh attempt
        import time
        time.sleep(20)
        res = run_bass_kernel_spmd(_CACHE["nc"], in_maps, core_ids=list(range(NCORES)))
    out = np.zeros((S, D), np.float64)
    for r in res.results:
        out += np.asarray(r["out"], np.float64)
    return out.astype(np.float32).reshape(B, S, D)


# revision 7
# speedup vs baseline: 1.1671x; 1.1211x over previous
"""GQA attention (int8-quantized QK^T, RoPE, causal softmax) on 8 TRN2 NeuronCores.

Sharding: tensor-parallel over heads. Core c owns Q heads 4c..4c+3 (Wq cols
512c..512c+512), KV head c (Wk/Wv cols 128c..128c+128), and Wo rows
512c..512c+512. x is replicated (host pre-transposes + casts to bf16). Each
core emits a partial [2048, 4096] bf16 output (its heads' contribution
through Wo); the host sums the 8 partials in float64. No on-device
collectives.

Per-core dataflow (all matmuls bf16; QK^T faithful to the reference's int8
quantization: integer values are produced with the fp32 round-to-even MAGIC
trick exactly as jnp.round does, then the dequant scales absmax/127 (and
SCALE for k) are folded back into the stored bf16 qT/kT, so the scores
matmul emits final logits with only bf16 representation noise ~0.2%):
  A) wkv first on the scalar HWDGE ring (the kv chain is the first PE
     consumer), then wq, then the x blocks prefetched two blocks ahead
     (triple-buffered) so they always land early; the first x block rides
     the idle sync ring in d-halves; cos/sin on gpsimd SWDGE. The sync ring
     otherwise carries ONLY the qi/ki transposes so they never back up
     behind bulk loads. KV projection before Q per s-tile; ScalarE
     evacuates PSUM; RoPE + absmax-quantize + scale-fold on VectorE;
     DMA-transpose q/k to [hd, s] on sync.
  B) per q-block J and head: scores^T [t, q] = kT-tile.T @ qT-block emit
     final logits; exp on ScalarE straight out of PSUM; causal zeroing of
     diagonal-band tiles on gpsimd post-exp; running probability sum on
     VectorE in bf16; den = ones.T @ sump matmul whose sump dependency is
     covered by held-back output-projection groups so the PE FIFO keeps
     streaming; O^T += V-chunk.T @ P^T.
  C) out[s, :] += OT-slice.T @ Wo-chunk accumulated over f, DMA out (bf16)
     per half-row on the Scalar queue. C(J) is emitted right after B(J) so
     its matmuls fill TensorE bubbles while B(J+1) waits on exp; ~8 groups
     per block are held back to cover the pair-end den/normalize chains.
"""

import numpy as np

import concourse.bass as bass
import concourse.mybir as mybir
import concourse.tile as tile
from concourse import bacc
from concourse.bass_utils import run_bass_kernel_spmd
from concourse.masks import make_identity

FP = mybir.dt.float32
FR = mybir.dt.float32r
BF = mybir.dt.bfloat16
AL = mybir.AluOpType
AF = mybir.ActivationFunctionType

B, S, D, NH, NKV, HD = 1, 2048, 4096, 32, 8, 128
NCORES = 8
HPC = NH // NCORES          # 4 Q heads per core
FQ = HPC * HD               # 512
SCALE = HD ** -0.5
MAGIC = 3 * 2.0 ** 22       # fp32 round-to-nearest-even magic constant

ST = S // 128               # 16 s-tiles of 128 rows
DC = D // 128               # 32 d-chunks
NJ = S // 512               # 4 q-blocks of 512
XBLK = 256                  # xT columns per DMA block
NXB = S // XBLK             # 8 blocks
NWARM = 96                  # HAM warm-up matmuls (~10us cold, covers wkv+xt0)


def build_graph():
    nc = bacc.Bacc(None)
    # all inputs arrive host-swizzled partition-major so every DMA descriptor
    # covers a multi-KB contiguous run (strided layouts fragment into 512B
    # descriptors and make the loads descriptor-bound, ~40us to first matmul)
    xt_e = nc.declare_dram_parameter("xt", [128, NXB * DC * XBLK], BF, isOutput=False)
    wq_e = nc.declare_dram_parameter("wq", [128, DC * FQ], BF, isOutput=False)
    wkv_e = nc.declare_dram_parameter("wkv", [128, DC * 2 * HD], BF, isOutput=False)
    wo_e = nc.declare_dram_parameter("wo", [128, HPC * D], BF, isOutput=False)
    cos_e = nc.declare_dram_parameter("cosr", [128, ST, HD], BF, isOutput=False)
    sin_e = nc.declare_dram_parameter("sinm", [128, ST, HD], BF, isOutput=False)
    out_e = nc.declare_dram_parameter("out", [S, D], BF, isOutput=True)

    xt_r = xt_e[:].rearrange("p (b c s) -> p b c s", b=NXB, c=DC)

    with tile.TileContext(nc, pool_alloc_mode="queue") as tc:
        with (
            tc.tile_pool(name="persist", bufs=1) as pp,
        ):
            ones1 = pp.tile([128, 1], BF)       # den stationary (M=1)
            nc.gpsimd.memset(ones1[:], 1.0)
            # warm the PE HAM clock gate with throwaway matmuls while the
            # first input DMAs land, so the real chains start at full clock
            warm = pp.tile([128, 128], BF)
            nc.gpsimd.memset(warm[:], 0.0)

            qT = pp.tile([128, HPC, S], BF)     # scale-folded Q^T per head
            kT = pp.tile([128, S], BF)          # scale-folded K^T (SCALE folded)
            vn = pp.tile([128, ST, HD], BF)     # V natural, per t-chunk
            OT = pp.tile([128, HPC, S], BF)     # normalized O^T per head
            wo_r = pp.tile([128, HPC, D], BF)

            # ---------------- Phase A: projections, RoPE, quantize+fold
            with (
                tc.tile_pool(name="aw", bufs=1) as awp,
                tc.tile_pool(name="xtp", bufs=2) as xtp,
                tc.tile_pool(name="ab", bufs=3) as ab,
                tc.tile_pool(name="psA", bufs=2, space="PSUM") as psA,
            ):
                wqr = awp.tile([128, DC, FQ], BF)
                wkv = awp.tile([128, DC, 2 * HD], BF)
                cosr = awp.tile([128, ST, HD], BF)
                sinm = awp.tile([128, ST, HD], BF)
                # startup-critical loads on three rings concurrently: wkv on
                # scalar, first x block on sync (d-halves), tables on gpsimd
                xtb_first = xtp.tile([128, DC, XBLK], BF, tag="xtb")
                nc.sync.dma_start(xtb_first[:, 0:16, :], xt_r[:, 0, 0:16])
                nc.sync.dma_start(xtb_first[:, 16:32, :], xt_r[:, 0, 16:32])
                nc.scalar.dma_start(wkv[:], wkv_e[:].rearrange("p (c h) -> p c h", c=DC))
                nc.gpsimd.dma_start(cosr[:], cos_e[:])
                nc.gpsimd.dma_start(sinm[:], sin_e[:])
                wq_r = wq_e[:].rearrange("p (c f) -> p c f", c=DC)
                for wc in range(4):
                    nc.scalar.dma_start(wqr[:, wc * 8:(wc + 1) * 8, :],
                                        wq_r[:, wc * 8:(wc + 1) * 8, :])
                # pre-warm the exp table set (one-time ~2.7us) after the
                # critical weight DMAs are already queued on ScalarE
                scratch = pp.tile([128, 1], FP)
                nc.scalar.activation(scratch[:], warm[:, 0:1], AF.Exp)

                # HAM warm-up: throwaway matmuls spanning the ~12us the first
                # input DMAs take to land, so the projection chains start warm
                wps = psA.tile([64, 128], FP, tag="wps")
                for w in range(NWARM):
                    nc.tensor.matmul(wps[:], warm[:, 0:64], warm[:],
                                     start=(w == 0), stop=(w == NWARM - 1))

                # x blocks 1..7 ride the scalar ring behind the weights,
                # prefetched one block ahead of their consumer chains so each
                # lands well before its projection chains need it
                xtbs = {0: xtb_first}

                def prefetch(blk):
                    if blk < NXB and blk not in xtbs:
                        t = xtp.tile([128, DC, XBLK], BF, tag="xtb")
                        nc.scalar.dma_start(t[:], xt_r[:, blk])
                        xtbs[blk] = t

                prefetch(1)

                for blk in range(NXB):
                    xtb = xtbs[blk]
                    prefetch(blk + 1)
                    if blk == 2:
                        # wo prefetch on the idle gpsimd ring (needed in C)
                        nc.gpsimd.dma_start(
                            wo_r[:], wo_e[:].rearrange("p (f d) -> p f d", f=HPC))

                    for i in range(XBLK // 128):
                        st_i = blk * (XBLK // 128) + i
                        xts = xtb[:, :, i * 128:(i + 1) * 128]
                        q_ps = psA.tile([128, FQ], FP, tag="qps")
                        kv_ps = psA.tile([128, 2 * HD], FP, tag="kvps")
                        # kv first: wkv is the first weight to land
                        for d in range(DC):
                            nc.tensor.matmul(kv_ps[:], xts[:, d, :], wkv[:, d, :],
                                             start=(d == 0), stop=(d == DC - 1))
                        for d in range(DC):
                            nc.tensor.matmul(q_ps[:], xts[:, d, :], wqr[:, d, :],
                                             start=(d == 0), stop=(d == DC - 1))

                        # ScalarE evacuates PSUM: V natural cast, q/k to fp32
                        nc.scalar.copy(vn[:, st_i, :], kv_ps[:, HD:2 * HD])
                        qf = ab.tile([128, HPC, HD], FP, tag="qf")
                        kf = ab.tile([128, 1, HD], FP, tag="kf")
                        nc.scalar.copy(kf[:], kv_ps[:, 0:HD].unsqueeze(1))
                        nc.scalar.copy(qf[:], q_ps[:].rearrange("p (h d) -> p h d", h=HPC))

                        # RoPE + quantize + scale-fold: q (4 heads) and k (1)
                        qi = ab.tile([128, HPC, HD], BF, tag="qi")
                        ki = ab.tile([128, 1, HD], BF, tag="ki")
                        co = cosr[:, st_i, :]
                        si = sinm[:, st_i, :]
                        for (src, nh, i8out, kscale) in (
                                (kf, 1, ki, SCALE), (qf, HPC, qi, 1.0)):
                            rr = ab.tile([128, nh, HD], FP, tag=f"rr{nh}")
                            t2 = ab.tile([128, nh, HD], FP, tag=f"t2{nh}")
                            am = ab.tile([128, nh], FP, tag=f"am{nh}")
                            am2 = ab.tile([128, nh], FP, tag=f"am2{nh}")
                            sc = ab.tile([128, nh], FP, tag=f"sc{nh}")
                            u = ab.tile([128, nh], FP, tag=f"u{nh}")
                            cob = co.unsqueeze(1).broadcast_to([128, nh, HD])
                            sib = si.unsqueeze(1).broadcast_to([128, nh, HD])
                            nc.vector.tensor_mul(rr[:], src[:], cob)
                            nc.vector.tensor_mul(t2[:, :, 0:64], src[:, :, 64:HD], sib[:, :, 0:64])
                            nc.vector.tensor_mul(t2[:, :, 64:HD], src[:, :, 0:64], sib[:, :, 64:HD])
                            nc.vector.tensor_add(rr[:], rr[:], t2[:])
                            nc.vector.tensor_reduce(am[:], rr[:], axis=mybir.AxisListType.X,
                                                    op=AL.max, apply_absolute_value=True)
                            nc.vector.tensor_scalar_max(am[:], am[:], 1e-5)
                            # am2 = am/127 ; sc = 127/am (Newton-refined)
                            nc.vector.tensor_scalar_mul(am2[:], am[:], 1.0 / 127.0)
                            nc.vector.reciprocal_approx_fast(sc[:], am2[:])
                            nc.vector.tensor_mul(u[:], am2[:], sc[:])
                            nc.vector.tensor_scalar(u[:], u[:], -1.0, 2.0, op0=AL.mult, op1=AL.add)
                            nc.vector.tensor_mul(sc[:], sc[:], u[:])
                            if kscale != 1.0:
                                amk = ab.tile([128, nh], FP, tag=f"amk{nh}")
                                nc.vector.tensor_scalar_mul(amk[:], am2[:], kscale)
                                unscale = amk
                            else:
                                unscale = am2
                            for h in range(nh):
                                nc.vector.tensor_scalar(rr[:, h, :], rr[:, h, :],
                                                        sc[:, h:h + 1], MAGIC,
                                                        op0=AL.mult, op1=AL.add)
                                nc.vector.tensor_scalar(i8out[:, h, :], rr[:, h, :],
                                                        MAGIC, unscale[:, h:h + 1],
                                                        op0=AL.subtract, op1=AL.mult)

                        # DMA-transpose folded q/k into [hd, s] layout on the
                        # sync queue (SBUF -> SBUF via the X-bar; the sync
                        # queue carries nothing else after the first x block,
                        # so these never back up behind bulk loads)
                        ssl = slice(st_i * 128, (st_i + 1) * 128)
                        nc.sync.dma_start(kT[:, ssl], ki[:, 0, :],
                                          transpose=True)
                        for h in range(HPC):
                            nc.sync.dma_start(qT[:, h, ssl], qi[:, h, :],
                                              transpose=True)

            # ---------------- Phases B+C interleaved per q-block J
            with (
                tc.tile_pool(name="bt", bufs=6) as bt,
                tc.tile_pool(name="bd", bufs=2) as bd,
                tc.tile_pool(name="ct", bufs=3) as ct,
                tc.tile_pool(name="psSC", bufs=3, space="PSUM") as psSC,
                tc.tile_pool(name="psO", bufs=1, space="PSUM") as psO,
                tc.tile_pool(name="psDen", bufs=1, space="PSUM") as psDen,
                tc.tile_pool(name="psC", bufs=2, space="PSUM") as psC,
            ):
                ct_state = {}

                def emit_c_group(J, k, ct_pool=None, psc_pool=None):
                    # one output-projection column group: 4 accumulating
                    # matmuls + one PSUM evacuation; DMA per (st, half)
                    ct_pool = ct_pool if ct_pool is not None else ct
                    psc_pool = psc_pool if psc_pool is not None else psC
                    st_i = 4 * J + k // 8
                    half = (k % 8) // 4
                    dbl = k % 4
                    db = half * 4 + dbl
                    ssl = slice(st_i * 128, (st_i + 1) * 128)
                    if dbl == 0:
                        ot_new = ct_pool.tile([128, D // 2], BF, tag="ot")
                        ct_state["ot"] = ot_new
                    ot_sb = ct_state["ot"]
                    wo_ps = psc_pool.tile([128, 512], FP, tag="wo")
                    for f in range(HPC):
                        nc.tensor.matmul(wo_ps[:], OT[:, f, ssl],
                                         wo_r[:, f, db * 512:(db + 1) * 512],
                                         start=(f == 0), stop=(f == HPC - 1))
                    if db % 4 == 0:
                        nc.scalar.copy(ot_sb[:, dbl * 512:(dbl + 1) * 512], wo_ps[:])
                    else:
                        nc.vector.tensor_copy(ot_sb[:, dbl * 512:(dbl + 1) * 512], wo_ps[:])
                    if dbl == 3:
                        nc.scalar.dma_start(
                            out_e[ssl, half * (D // 2):(half + 1) * (D // 2)],
                            ot_sb[:])

                for J in range(NJ):
                    nlive = 4 * J + 4
                    Jsl = slice(J * 512, (J + 1) * 512)
                    # interleave the previous q-block's output projection into
                    # this block's attention loop: the PE engine FIFO is strict,
                    # so C matmuls must be emitted inside B's exp-paced stretches
                    # to fill them; ~8 groups are held back for the pair-end
                    # den/normalize chains
                    c_queue = list(range(32)) if J > 0 else []
                    n_iters = 2 * nlive
                    it = 0
                    emitted = 0

                    def emit_some(n):
                        nonlocal emitted
                        for _ in range(n):
                            if not c_queue:
                                return
                            emit_c_group(J - 1, c_queue.pop(0))
                            emitted += 1

                    # diagonal-band tiles first: their gpsimd causal selects
                    # pipeline against the clean tiles that follow instead of
                    # stalling the accumulation tail
                    ti_order = list(range(4 * J, nlive)) + list(range(0, 4 * J))
                    for hp in range(HPC // 2):
                        # two single-head streams zipped at tile granularity:
                        # one head's matmuls hide the other's exp round-trip
                        # latency without needing paired PSUM tiles
                        h0, h1 = 2 * hp, 2 * hp + 1
                        oT0 = psO.tile([128, 512], FP, tag="o0")
                        oT1 = psO.tile([128, 512], FP, tag="o1")
                        # bf16 probability accumulators: 2x DVE rate; the
                        # ~0.3% bf16 accumulation noise on den is well within
                        # the error budget
                        sump0 = bd.tile([128, 512], BF, tag="sump0")
                        sump1 = bd.tile([128, 512], BF, tag="sump1")
                        for idx, ti in enumerate(ti_order):
                            # columns below off are fully above the causal
                            # diagonal for this tile; skip them everywhere
                            off = max(0, ti * 128 - J * 512)
                            kts = kT[:, ti * 128:(ti + 1) * 128]
                            qsl = slice(J * 512 + off, (J + 1) * 512)
                            pts = []
                            for g, hh in ((0, h0), (1, h1)):
                                sc_ps = psSC.tile([128, 512], FP, tag="sc")
                                nc.tensor.matmul(sc_ps[:, off:], kts, qT[:, hh, qsl])
                                pt = bt.tile([128, 512], BF, tag="pt")
                                nc.scalar.activation(pt[:, off:], sc_ps[:, off:], AF.Exp)
                                if ti >= 4 * J:
                                    nc.gpsimd.affine_select(
                                        out=pt[:, off:], in_=pt[:, off:],
                                        compare_op=AL.is_ge, fill=0.0,
                                        base=0, channel_multiplier=-1,
                                        pattern=[[1, 512 - off]])
                                pts.append(pt)
                            for (pt, oT, sump) in ((pts[0], oT0, sump0),
                                                   (pts[1], oT1, sump1)):
                                nc.tensor.matmul(oT[:, off:], vn[:, ti, :], pt[:, off:],
                                                 start=(idx == 0), stop=(idx == nlive - 1))
                                if idx == 0:
                                    nc.vector.tensor_copy(sump[:], pt[:])
                                else:
                                    nc.vector.tensor_add(sump[:, off:], sump[:, off:],
                                                         pt[:, off:])
                            it += 1
                            while c_queue and emitted < it * 24 // n_iters:
                                emit_c_group(J - 1, c_queue.pop(0))
                                emitted += 1
                        # pair-end den/normalize: cover the sump->den matmul
                        # dependency and the den-bank recycle with held-back C
                        # groups so the PE FIFO keeps streaming
                        emit_some(2)
                        for (oT, sump, hh) in ((oT0, sump0, h0), (oT1, sump1, h1)):
                            den_ps = psDen.tile([1, 512], FP, tag="den")
                            nc.tensor.matmul(den_ps[:], ones1[:], sump[:])
                            emit_some(1)
                            denr = bd.tile([1, 512], FP, tag="denr")
                            nc.vector.reciprocal_approx_fast(denr[:], den_ps[:])
                            dnb = bd.tile([128, 512], FP, tag="dnb")
                            nc.gpsimd.partition_broadcast(dnb[:], denr[:])
                            nc.vector.tensor_mul(OT[:, hh, Jsl], oT[:], dnb[:])
                        emit_some(1)

                    # drain any of last block's C groups the cadence missed
                    while c_queue:
                        emit_c_group(J - 1, c_queue.pop(0))

            # final q-block's output projection: fresh PSUM scope (the B pools
            # are dead now) so it can deep-buffer and run at full PE rate
            with (
                tc.tile_pool(name="ct2", bufs=3) as ct2,
                tc.tile_pool(name="psC2", bufs=4, space="PSUM") as psC2,
            ):
                for k in range(32):
                    emit_c_group(NJ - 1, k, ct_pool=ct2, psc_pool=psC2)

    nc.compile()
    return nc


def _pmajor(w):
    # [K*128, N] -> [128, K*N] with each partition's K rows contiguous
    k128, n = w.shape
    return np.ascontiguousarray(
        w.reshape(k128 // 128, 128, n).transpose(1, 0, 2).reshape(128, -1))


def make_in_maps(x, Wq, Wk, Wv, Wo, cos, sin):
    import ml_dtypes
    bf = ml_dtypes.bfloat16
    x2 = np.asarray(x, np.float32).reshape(S, D).astype(bf)
    # [p, blk, c, s_local] so each xtb block is 16KB contiguous per partition
    xt = np.ascontiguousarray(
        x2.reshape(NXB, XBLK, DC, 128).transpose(3, 0, 2, 1).reshape(128, -1))
    cosr = np.ascontiguousarray(
        np.asarray(cos, np.float32).reshape(ST, 128, HD).transpose(1, 0, 2).astype(bf))
    sinm_f = np.asarray(sin, np.float32).copy()
    sinm_f[:, :64] *= -1.0
    sinm = np.ascontiguousarray(
        sinm_f.reshape(ST, 128, HD).transpose(1, 0, 2).astype(bf))
    Wq = np.asarray(Wq, np.float32)
    Wk = np.asarray(Wk, np.float32)
    Wv = np.asarray(Wv, np.float32)
    Wo = np.asarray(Wo, np.float32)
    in_maps = []
    for c in range(NCORES):
        wkv = np.concatenate(
            [Wk[:, c * HD:(c + 1) * HD], Wv[:, c * HD:(c + 1) * HD]], axis=1)
        in_maps.append({
            "xt": xt,
            "wq": _pmajor(Wq[:, c * FQ:(c + 1) * FQ].astype(bf)),
            "wkv": _pmajor(wkv.astype(bf)),
            "wo": _pmajor(Wo[c * FQ:(c + 1) * FQ, :].astype(bf)),
            "cosr": cosr,
            "sinm": sinm,
        })
    return in_maps


_CACHE = {}


def kernel(x, Wq, Wk, Wv, Wo, cos, sin):
    in_maps = make_in_maps(x, Wq, Wk, Wv, Wo, cos, sin)
    if "nc" not in _CACHE:
        _CACHE["nc"] = build_graph()
    try:
        res = run_bass_kernel_spmd(_CACHE["nc"], in_maps, core_ids=list(range(NCORES)))
    except Exception:
        # transient NRT/device hiccups (e.g. EXEC_UNIT_UNRECOVERABLE) usually
        # clear on a fresh attempt
        import time
        time.sleep(20)
        res = run_bass_kernel_spmd(_CACHE["nc"], in_maps, core_ids=list(range(NCORES)))
    out = np.zeros((S, D), np.float64)
    for r in res.results:
        out += np.asarray(r["out"], np.float64)
    return out.astype(np.float32).reshape(B, S, D)
